# revision 50
# baseline (speedup 1.0000x reference)
"""Trainium2 Bass kernel for a dense transformer block (B=64,T=256,C=1024,H=16).

Sharding: pure data-parallel over batch across 8 NeuronCores (8 sequences
per core, no collectives). Per-core program:
  phase A (per batch PAIR): LN1 (bn_stats, x ships bf16) -> h ->
  PE-transpose -> hT as bf16 (for V) and 16x-scaled fp8 k-pairs (for
  q/k); q/k via fp8 DoubleRow matmuls over both batches (N=512, PSUM
  drained on the scalar engine), V/proj in bf16; causal attention with
  both score blocks of a head packed into one PSUM bank; V stored
  head-interleaved as [V_h | 1] 65-col blocks so attn@V's 65th output
  column IS the softmax denominator (no N=1 ones-matmuls); exp on
  scalar, diag masks on vector (e0) / gpsimd (e1); normalization via
  per-partition vector multiplies -> PE-transpose cat back to [c,t] ->
  proj -> residual -> spill x2 to DRAM (batch 0 stays SBUF-resident).
  phase B (per 256-token supertile): LN2 -> yT emitted directly as
  16x-scaled fp8 k-pairs -> z1 = relu(w1.T y / 256) with w1 in fp8
  DoubleRow (halves GEMM1's PE passes; error budget: ~1.1e-2
  scale-relative absmax vs the 2e-2 gate; a both-GEMM fp8 FFN lands
  ~1.5e-2 - too close) -> z2 accumulation in bf16 (N=512 matmuls)
  -> LN3 -> final residual -> out.
  A top-level "bridge" pool holds supertile 0's x2/LN2/yT-fp8 and the
  first FFN weight chunk, all produced during phase A, so phase B's
  first pz matmul issues the moment the phase-A PSUM pool releases
  (pool-boundary serialization otherwise costs ~12us); the remaining
  w1 streams in fb-octet column chunks interleaved with the w2 rows
  they pair with.  LN prep and input loads software-pipeline one
  iteration ahead in both phases.
"""

from contextlib import ExitStack

import ml_dtypes
import numpy as np

import concourse.bass as bass
import concourse.bacc as bacc
import concourse.mybir as mybir
import concourse.tile as tile
from concourse.bass_utils import run_bass_kernel_spmd

F32 = mybir.dt.float32
BF16 = mybir.dt.bfloat16
F8E4 = mybir.dt.float8e4
AF = mybir.ActivationFunctionType
ALU = mybir.AluOpType
AX = mybir.AxisListType

B, T, C, H, D = 64, 256, 1024, 16, 64
FP8_FFN = True            # FFN first GEMM in fp8 DoubleRow (2x PE rate)
NCORES = 8
NB = B // NCORES          # 8 sequences per core
TOK = NB * T              # 2048 tokens per core
F4 = 4 * C                # 4096
EPS = 1e-3
SCALE = C ** -0.5         # 1/32

_CACHE = {}


def _ln_tile(nc, pools, xt, out_t, affine, ncols=C, scale32=False,
             ts_eng=None):
    """LayerNorm of one [128, ncols] tile along the free axis via bn_stats.
    out_t may equal xt (in-place). affine = (g_t, be_t) or None.
    scale32: emit 32*(x-m)*rstd by folding 1/1024 into the sqrt.
    ts_eng: engine for the big normalize pass (e.g. nc.gpsimd when both
    xt and out_t are SBUF tiles); defaults to the vector engine."""
    stat = pools["stat"]
    nsub = ncols // 512
    st = stat.tile([128, nsub, 6], F32, tag="bst", name="bst")
    mv = stat.tile([128, 2], F32, tag="bmv", name="bmv")
    rs = stat.tile([128, 1], F32, tag="brs", name="brs")
    xv = xt[:].rearrange("p (a b) -> p a b", b=512)
    for i in range(nsub):
        nc.vector.bn_stats(st[:, i, :], xv[:, i, :])
    nc.vector.bn_aggr(mv[:], st[:])
    if scale32:
        nc.scalar.activation(rs[:], mv[:, 1:2], AF.Sqrt,
                             bias=pools["eps1024"][:], scale=1.0 / 1024.0)
    else:
        nc.scalar.activation(rs[:], mv[:, 1:2], AF.Sqrt, bias=pools["eps"][:])
    nc.vector.reciprocal(rs[:], rs[:])
    (ts_eng or nc.vector).tensor_scalar(out_t[:], xt[:], mv[:, 0:1], rs[:],
                                        ALU.subtract, ALU.mult)
    if affine is not None:
        g_t, be_t = affine
        nc.vector.tensor_tensor(out_t[:], out_t[:], g_t[:], ALU.mult)
        nc.vector.tensor_tensor(out_t[:], out_t[:], be_t[:], ALU.add)


def _build(flags):
    aff1, aff2, aff3, use_bproj, use_b1, use_b2 = flags
    nc = bacc.Bacc(target_bir_lowering=False)
    # x ships bf16: halves the xb pool + startup DMA; the trunk residual
    # only carries bf16 rounding of x (~4x below the fp8-FFN error).
    x_d = nc.dram_tensor("x", [TOK, C], BF16, kind="ExternalInput")
    # q/k projections ship as fp8e4, pre-scaled by 16 and packed in the
    # DoubleRow k-pair layout [C/2 rows, 2*C cols]; the 16*16*16*16 scale
    # surplus is divided out inside the softmax exp.
    wqv_d = nc.dram_tensor("wqv", [C // 2, 2 * C], F8E4, kind="ExternalInput")
    wkv_d = nc.dram_tensor("wkv", [C // 2, 2 * C], F8E4, kind="ExternalInput")
    wv_d = nc.dram_tensor("wvf", [C, C], BF16, kind="ExternalInput")
    wp_d = nc.dram_tensor("wpf", [C, C], BF16, kind="ExternalInput")
    if FP8_FFN:
        # w1 ships fp8e4 pre-scaled by 16 in DoubleRow k-pair layout,
        # like wq/wk; the 16*16 scale is divided out in the relu.
        w1_d = nc.dram_tensor("w1q", [C // 2, 2 * F4], F8E4,
                              kind="ExternalInput")
    else:
        w1_d = nc.dram_tensor("w1f", [C, F4], BF16, kind="ExternalInput")
    w2_d = nc.dram_tensor("w2f", [F4, C], BF16, kind="ExternalInput")
    consts_bf = {}
    names = []
    if use_b1:
        b1_d = nc.dram_tensor("b1t", [128, F4 // 128], F32, kind="ExternalInput")
    if use_bproj:
        names.append("bprojb")
    if use_b2:
        names.append("b2b")
    if aff1:
        names += ["g1b", "be1b"]
    if aff2:
        names += ["g2b", "be2b"]
    if aff3:
        names += ["g3b", "be3b"]
    for nm in names:
        consts_bf[nm] = nc.dram_tensor(nm, [128, C], BF16, kind="ExternalInput")
    m0_d = nc.dram_tensor("mask0", [128, 128], BF16, kind="ExternalInput")
    id_d = nc.dram_tensor("identb", [128, 128], BF16, kind="ExternalInput")
    out_d = nc.dram_tensor("out", [TOK, C], F32, kind="ExternalOutput")
    x2_d = nc.dram_tensor("x2d", [TOK, C], F32)

    with tile.TileContext(nc) as tc, ExitStack() as ctx:
        const = ctx.enter_context(tc.tile_pool(name="const", bufs=1))
        cb_t = {nm: const.tile([128, C], BF16, tag=nm, name=nm)
                for nm in consts_bf}
        for nm, t in cb_t.items():
            nc.sync.dma_start(out=t[:], in_=consts_bf[nm][:, :])
        m0 = const.tile([128, 128], BF16, tag="m0", name="m0")
        nc.sync.dma_start(out=m0[:], in_=m0_d[:, :])
        idb = const.tile([128, 128], BF16, tag="idb", name="idb")
        nc.sync.dma_start(out=idb[:], in_=id_d[:, :])
        if use_b1:
            b1t = const.tile([128, F4 // 128], F32, tag="b1t", name="b1t")
            nc.sync.dma_start(out=b1t[:], in_=b1_d[:, :])

        epsb = const.tile([128, 1], F32, tag="eps", name="eps")
        nc.gpsimd.memset(epsb[:], EPS)
        epsb2 = const.tile([128, 1], F32, tag="eps2", name="eps2")
        nc.gpsimd.memset(epsb2[:], EPS / 1024.0)

        stat = ctx.enter_context(tc.tile_pool(name="stat", bufs=8))
        pools = {"stat": stat, "eps": epsb, "eps1024": epsb2}

        # bridge pool: first FFN weight chunk lives at top level so its DMA
        # streams during phase A (phase-B pools can't allocate until the
        # phase-A pools release, serializing their DMAs behind all of A).
        bridge = ctx.enter_context(tc.tile_pool(name="bridge", bufs=1))
        w1_oct0, w2_pre = [], []
        if FP8_FFN:
            for cbp in range(4):
                t = bridge.tile([128, 2 * 1024], F8E4, tag=f"w1o0_{cbp}",
                                name=f"w1o0_{cbp}")
                w1_oct0.append(t)
        for fb in range(2):
            t = bridge.tile([128, C], BF16, tag=f"w2p{fb}", name=f"w2p{fb}")
            w2_pre.append(t)
        # supertile 0's x2 / LN2 / yT(fp8) also live in the bridge: its FFN
        # inputs are fully prepared inside phase A, so the first pz matmul
        # issues the moment the phase-A PSUM pool releases.
        bridgeB = {
            "x2": [bridge.tile([128, C], F32, tag=f"bx2_{tb}", name=f"bx2_{tb}")
                   for tb in range(2)],
            "ybf": [bridge.tile([128, C], BF16, tag=f"by_{tb}", name=f"by_{tb}")
                    for tb in range(2)],
            "yf8": [bridge.tile([128, 2 * 256], F8E4, tag=f"by8_{c}",
                                name=f"by8_{c}") for c in range(4)],
        }

        # ---------------- phase A: attention ----------------
        with ExitStack() as actx:
            xb_p = actx.enter_context(tc.tile_pool(name="xb", bufs=12))
            xb_tiles = {}

            def load_x(b):
                if b >= NB:
                    return
                ts = [xb_p.tile([128, C], BF16, tag="xb", name="xb")
                      for _ in range(2)]
                for tb in range(2):
                    row = b * T + tb * 128
                    # halves: LN1's first bn_stats starts at half-tile
                    for hh in range(2):
                        cs = slice(hh * 512, (hh + 1) * 512)
                        nc.sync.dma_start(out=ts[tb][:, cs],
                                          in_=x_d[row:row + 128, cs])
                xb_tiles[b] = ts

            # first pair's input loads go on the queue before the weight
            # DMAs so LN1 can start while weights stream in; pair 1's loads
            # follow the weights (they aren't needed until ~30us in).
            load_x(0)
            load_x(1)

            wpool = actx.enter_context(tc.tile_pool(name="wqkv", bufs=1))
            wqv_sb, wkv_sb = [], []
            for cbp in range(4):
                for lst, dram, nm in ((wqv_sb, wqv_d, "wqv"),
                                      (wkv_sb, wkv_d, "wkv")):
                    t = wpool.tile([128, 2 * C], F8E4, tag=f"{nm}{cbp}",
                                   name=f"{nm}{cbp}")
                    nc.sync.dma_start(out=t[:],
                                      in_=dram[cbp * 128:(cbp + 1) * 128, :])
                    lst.append(t)
            wv_sb, wp_sb = [], []
            for cb in range(8):
                for lst, dram, nm in ((wv_sb, wv_d, "wv"), (wp_sb, wp_d, "wp")):
                    t = wpool.tile([128, C], BF16, tag=f"{nm}{cb}", name=f"{nm}{cb}")
                    nc.sync.dma_start(out=t[:], in_=dram[cb * 128:(cb + 1) * 128, :])
                    lst.append(t)
            load_x(2)
            load_x(3)
            h_p = actx.enter_context(tc.tile_pool(name="h", bufs=4))
            ht_p = actx.enter_context(tc.tile_pool(name="ht", bufs=16))
            hf8_p = actx.enter_context(tc.tile_pool(name="hf8", bufs=8))
            qt_p = actx.enter_context(tc.tile_pool(name="qt", bufs=12))
            kt_p = actx.enter_context(tc.tile_pool(name="kt", bufs=12))
            v_p = actx.enter_context(tc.tile_pool(name="v", bufs=8))
            ex_p = actx.enter_context(tc.tile_pool(name="ex", bufs=8))
            cat_p = actx.enter_context(tc.tile_pool(name="cat", bufs=4))
            ctt_p = actx.enter_context(tc.tile_pool(name="ctt", bufs=10))
            rec_p = actx.enter_context(tc.tile_pool(name="rec", bufs=12))
            x2_p = actx.enter_context(tc.tile_pool(name="x2", bufs=4))
            ps = actx.enter_context(tc.tile_pool(name="psA", bufs=8, space="PSUM"))

            prepped = {}

            def prep_A2(bp):
                """LN1 + h-transpose for a batch pair (emitted one pair early
                so the in-order vector/PE queues overlap it with pair bp-1).
                Produces hT both as bf16 (for V) and as 16x-scaled fp8 pairs
                (for the DoubleRow q/k matmuls)."""
                if bp >= NB // 2:
                    return
                b0, b1 = 2 * bp, 2 * bp + 1
                xbs = {b0: xb_tiles.pop(b0), b1: xb_tiles.pop(b1)}
                hbf = [h_p.tile([128, C], BF16, tag="h", name="h")
                       for _ in range(4)]
                for i in range(4):
                    _ln_tile(nc, pools, xbs[b0 if i < 2 else b1][i % 2],
                             hbf[i],
                             (cb_t["g1b"], cb_t["be1b"]) if aff1 else None)
                ht, hf8 = [], []
                for cbp in range(4):
                    hf8.append(hf8_p.tile([128, 2 * 512], F8E4, tag="hf8",
                                          name="hf8"))
                for cb in range(8):
                    pt = ps.tile([128, 512], BF16, tag="ps", name="ps")
                    for tt in range(4):
                        nc.tensor.transpose(
                            pt[:, tt * 128:(tt + 1) * 128],
                            hbf[tt][:, cb * 128:(cb + 1) * 128], idb[:])
                    t = ht_p.tile([128, 512], BF16, tag="ht", name="ht")
                    nc.vector.tensor_copy(t[:], pt[:])
                    ht.append(t)
                    nc.scalar.mul(
                        hf8[cb // 2][:, (cb % 2) * 512:(cb % 2) * 512 + 512],
                        pt[:], 16.0)
                prepped[bp] = (xbs, ht, hf8)

            prep_A2(0)

            for bp in range(NB // 2):
                xbs, ht, hf8 = prepped.pop(bp)
                hf8v = [t[:].rearrange("p (j n) -> p j n", j=2) for t in hf8]
                # q/k for both batches: fp8 DoubleRow, 256x-scaled outputs
                qt, kt = [], []
                for p in range(8):
                    pq = ps.tile([128, 512], F32, tag="ps", name="ps")
                    pk = ps.tile([128, 512], F32, tag="ps", name="ps")
                    for cbp in range(4):
                        wqs = wqv_sb[cbp][:].rearrange(
                            "p (j n) -> p j n", j=2)[:, :, p * 128:(p + 1) * 128]
                        wks = wkv_sb[cbp][:].rearrange(
                            "p (j n) -> p j n", j=2)[:, :, p * 128:(p + 1) * 128]
                        nc.tensor.matmul(
                            pq[:], wqs, hf8v[cbp],
                            perf_mode=mybir.MatmulPerfMode.DoubleRow,
                            start=(cbp == 0), stop=(cbp == 3))
                        nc.tensor.matmul(
                            pk[:], wks, hf8v[cbp],
                            perf_mode=mybir.MatmulPerfMode.DoubleRow,
                            start=(cbp == 0), stop=(cbp == 3))
                    tq = qt_p.tile([128, 512], BF16, tag="qt", name="qt")
                    tk = kt_p.tile([128, 512], BF16, tag="kt", name="kt")
                    # kt drains on scalar: vector is the scarce engine during
                    # the attention/prep overlap windows (qt stays on vector;
                    # both on scalar starves the exp stream).
                    nc.vector.tensor_copy(tq[:], pq[:])
                    nc.scalar.copy(tk[:], pk[:])
                    qt.append(tq)
                    kt.append(tk)
                # V per batch (bf16), stored head-interleaved as [V_h | 1]
                # blocks of 65 cols so attn@V emits the softmax denominator
                # as a fused 65th output column (kills the N=1 ones-matmuls).
                vsbs = {}
                for bi, b in enumerate((2 * bp, 2 * bp + 1)):
                    vsb = []
                    for sb in range(2):
                        scol = (bi * 2 + sb) * 128
                        pv = [ps.tile([128, 512], F32, tag="ps", name="ps")
                              for _ in range(2)]
                        for cb in range(8):
                            for q4 in range(4):
                                nc.tensor.matmul(
                                    pv[q4 // 2][:, (q4 % 2) * 256:(q4 % 2) * 256 + 256],
                                    ht[cb][:, scol:scol + 128],
                                    wv_sb[cb][:, q4 * 256:(q4 + 1) * 256],
                                    start=(cb == 0 and q4 % 2 == 0),
                                    stop=(cb == 7 and q4 % 2 == 1),
                                    skip_group_check=True)
                        tv = v_p.tile([128, 16 * 65], BF16, tag="v", name="v")
                        tv3 = tv[:].rearrange("p (h x) -> p h x", x=65)
                        nc.gpsimd.memset(tv3[:, :, 64], 1.0)
                        nc.vector.tensor_copy(
                            tv3[:, 0:8, 0:64],
                            pv[0][:].rearrange("p (h x) -> p h x", x=64))
                        nc.vector.tensor_copy(
                            tv3[:, 8:16, 0:64],
                            pv[1][:].rearrange("p (h x) -> p h x", x=64))
                        vsb.append(tv)
                    vsbs[b] = vsb
                # attention + proj per batch
                for bi, b in enumerate((2 * bp, 2 * bp + 1)):
                    vsb = vsbs[b]
                    bcol = bi * 256
                    cat_t = [cat_p.tile([128, C], BF16, tag="cat", name="cat")
                             for _ in range(2)]
                    for pr in range(8):
                        if bi == 1 and pr == 2:
                            load_x(2 * bp + 4)
                            load_x(2 * bp + 5)
                            prep_A2(bp + 1)
                        # both score blocks of one head packed in one PSUM
                        # bank (cols 0:256 = s-block0 x both tb, 256:384 =
                        # s-block1 x tb1) -> 3 ring slots per pr instead of 5.
                        scps, e0s, e1s = [], [], []
                        for off in (0, 64):
                            qs = qt[pr][off:off + 64, bcol:bcol + 256]
                            ks = kt[pr][off:off + 64, bcol:bcol + 256]
                            scp = ps.tile([128, 384], F32, tag="ps", name="scp")
                            nc.tensor.matmul(scp[:, 0:256], ks[:, 0:128],
                                             qs[:], start=True, stop=False,
                                             skip_group_check=True)
                            nc.tensor.matmul(scp[:, 256:384], ks[:, 128:256],
                                             qs[:, 128:256], start=False,
                                             stop=True, skip_group_check=True)
                            scps.append(scp)
                        for i in range(2):
                            e0 = ex_p.tile([128, 256], BF16, tag="e0", name="e0")
                            e1 = ex_p.tile([128, 128], BF16, tag="e1", name="e1")
                            nc.scalar.activation(e0[:], scps[i][:, 0:256],
                                                 AF.Exp, scale=SCALE / 65536.0)
                            nc.scalar.activation(e1[:], scps[i][:, 256:384],
                                                 AF.Exp, scale=SCALE / 65536.0)
                            # e0's mask feeds the first attn@V matmul: keep it
                            # on vector (short latency); e1 has more slack and
                            # keeps gpsimd from idling.
                            nc.vector.tensor_tensor(e0[:, 0:128], e0[:, 0:128],
                                                    m0[:], ALU.mult)
                            nc.gpsimd.tensor_tensor(e1[:], e1[:], m0[:],
                                                    ALU.mult)
                            e0s.append(e0)
                            e1s.append(e1)
                        # attn@V for BOTH heads packed in one bank as four
                        # 65-col blocks [out_h | den]: the denominator rides
                        # along as V's interleaved ones column.  One
                        # accumulation chain across all 6 matmuls.
                        att = ps.tile([128, 260], F32, tag="ps", name="att")
                        for i in range(2):
                            hh = 2 * pr + i
                            hs65 = slice(hh * 65, hh * 65 + 65)
                            e0, e1 = e0s[i], e1s[i]
                            o = 130 * i
                            nc.tensor.matmul(att[:, o:o + 65], e0[:, 0:128],
                                             vsb[0][:, hs65], start=(i == 0),
                                             stop=False, skip_group_check=True)
                            nc.tensor.matmul(att[:, o + 65:o + 130],
                                             e0[:, 128:256], vsb[0][:, hs65],
                                             start=False, stop=False,
                                             skip_group_check=True)
                            nc.tensor.matmul(att[:, o + 65:o + 130], e1[:],
                                             vsb[1][:, hs65], start=False,
                                             stop=(i == 1),
                                             skip_group_check=True)
                        # one reciprocal over all four denominators (strided
                        # col 64 of each 65-block): RAW on the last matmul of
                        # the bank, so the norm muls follow all PE writes.
                        av = att[:].rearrange("p (a b) -> p a b", b=65)
                        rec = rec_p.tile([128, 4], F32, tag="rec", name="rec")
                        nc.vector.reciprocal(rec[:], av[:, :, 64])
                        for i in range(2):
                            hh = 2 * pr + i
                            hs = slice(hh * 64, (hh + 1) * 64)
                            o = 130 * i
                            nc.vector.tensor_scalar_mul(
                                cat_t[0][:, hs], att[:, o:o + 64],
                                rec[:, 2 * i:2 * i + 1])
                            nc.vector.tensor_scalar_mul(
                                cat_t[1][:, hs], att[:, o + 65:o + 129],
                                rec[:, 2 * i + 1:2 * i + 2])
                    # transpose cat_t -> catT [c, t]
                    catT = []
                    for cb in range(8):
                        pt = ps.tile([128, 256], BF16, tag="ps", name="ps")
                        for tb in range(2):
                            nc.tensor.transpose(
                                pt[:, tb * 128:(tb + 1) * 128],
                                cat_t[tb][:, cb * 128:(cb + 1) * 128], idb[:])
                        t = ctt_p.tile([128, 256], BF16, tag="ctt", name="ctt")
                        nc.vector.tensor_copy(t[:], pt[:])
                        catT.append(t)
                    # proj + residual -> x2 -> DRAM spill (batch 0 keeps its
                    # x2 SBUF-resident in the bridge; no round-trip).
                    xb = xbs[b]
                    for tb in range(2):
                        if b == 0:
                            x2t = bridgeB["x2"][tb]
                        else:
                            x2t = x2_p.tile([128, C], F32, tag="x2", name="x2")
                        for n in range(2):
                            pp = ps.tile([128, 512], F32, tag="ps", name="ps")
                            for cb in range(8):
                                for nh in range(2):
                                    nc.tensor.matmul(
                                        pp[:, nh * 256:(nh + 1) * 256],
                                        catT[cb][:, tb * 128:(tb + 1) * 128],
                                        wp_sb[cb][:, n * 512 + nh * 256:
                                                   n * 512 + (nh + 1) * 256],
                                        start=(cb == 0 and nh == 0),
                                        stop=(cb == 7 and nh == 1),
                                        skip_group_check=True)
                            nsl = slice(n * 512, (n + 1) * 512)
                            nc.vector.tensor_tensor(x2t[:, nsl], pp[:],
                                                    xb[tb][:, nsl], ALU.add)
                            if use_bproj:
                                nc.vector.tensor_tensor(
                                    x2t[:, nsl], x2t[:, nsl],
                                    cb_t["bprojb"][:, nsl], ALU.add)
                        if b != 0:
                            row = b * T + tb * 128
                            nc.sync.dma_start(out=x2_d[row:row + 128, :],
                                              in_=x2t[:])

            # bridge weight DMAs: emitted last so they yield DMA priority to
            # phase A's own traffic, but stream well before the boundary.
            if FP8_FFN:
                for cbp in range(4):
                    for j in range(2):
                        nc.sync.dma_start(
                            out=w1_oct0[cbp][:, j * 1024:(j + 1) * 1024],
                            in_=w1_d[cbp * 128:(cbp + 1) * 128,
                                     j * F4:j * F4 + 1024])
            for fb in range(2):
                nc.sync.dma_start(out=w2_pre[fb][:],
                                  in_=w2_d[fb * 128:(fb + 1) * 128, :])

            # prep supertile 0's FFN inputs inside phase A (LN2 on vector,
            # transposes through the phase-A PSUM ring, fp8 via scalar).
            for tb in range(2):
                _ln_tile(nc, pools, bridgeB["x2"][tb], bridgeB["ybf"][tb],
                         (cb_t["g2b"], cb_t["be2b"]) if aff2 else None)
            for cb in range(8):
                pt0 = ps.tile([128, 256], BF16, tag="ps", name="pt0")
                for tb in range(2):
                    nc.tensor.transpose(
                        pt0[:, tb * 128:(tb + 1) * 128],
                        bridgeB["ybf"][tb][:, cb * 128:(cb + 1) * 128], idb[:])
                nc.scalar.mul(
                    bridgeB["yf8"][cb // 2][:, (cb % 2) * 256:(cb % 2) * 256 + 256],
                    pt0[:], 16.0)

        # ---------------- phase B: FFN ----------------
        with ExitStack() as bctx:
            x2B_p = bctx.enter_context(tc.tile_pool(name="x2B", bufs=12))
            x2_tiles = {}

            def load_x2(stx):
                # stx 0 is SBUF-resident via the bridge; guard duplicates
                # (the steady-state prefetch revisits early indices).
                if stx >= NB or stx == 0 or stx in x2_tiles or stx in _x2_seen:
                    return
                _x2_seen.add(stx)
                ts = [x2B_p.tile([128, C], F32, tag="x2B", name="x2B")
                      for _ in range(2)]
                for tb in range(2):
                    row = stx * 256 + tb * 128
                    nc.sync.dma_start(out=ts[tb][:], in_=x2_d[row:row + 128, :])
                x2_tiles[stx] = ts

            _x2_seen = set()
            # first supertiles' loads precede the FFN weight DMAs on the queue
            load_x2(1)
            load_x2(2)

            wpoolB = bctx.enter_context(tc.tile_pool(name="wffn", bufs=1))
            # w1q lives as per-fb-octet tiles: octet 0 is the bridge pool's
            # (DMA'd during phase A); octets 1-3 stream here, interleaved
            # with the w2 rows they pair with.
            w1_oct, w2_sb = [w1_oct0], list(w2_pre)
            for fb in range(2, 32):
                t = wpoolB.tile([128, C], BF16, tag=f"w2_{fb}", name=f"w2_{fb}")
                w2_sb.append(t)
            if FP8_FFN:
                for oc in range(1, 4):
                    tiles = []
                    for cbp in range(4):
                        t = wpoolB.tile([128, 2 * 1024], F8E4,
                                        tag=f"w1o{oc}_{cbp}",
                                        name=f"w1o{oc}_{cbp}")
                        tiles.append(t)
                    w1_oct.append(tiles)
                for fb in range(2, 8):
                    nc.sync.dma_start(out=w2_sb[fb][:],
                                      in_=w2_d[fb * 128:(fb + 1) * 128, :])
                for oc in range(1, 4):
                    for cbp in range(4):
                        for j in range(2):
                            nc.sync.dma_start(
                                out=w1_oct[oc][cbp][:, j * 1024:(j + 1) * 1024],
                                in_=w1_d[cbp * 128:(cbp + 1) * 128,
                                         j * F4 + oc * 1024:
                                         j * F4 + (oc + 1) * 1024])
                    for fb in range(oc * 8, (oc + 1) * 8):
                        nc.sync.dma_start(out=w2_sb[fb][:],
                                          in_=w2_d[fb * 128:(fb + 1) * 128, :])
            else:
                w1_sb = []
                for cb in range(8):
                    t = wpoolB.tile([128, F4], BF16, tag=f"w1_{cb}",
                                    name=f"w1_{cb}")
                    w1_sb.append(t)
                for ch in range(4):
                    cs = slice(ch * 1024, (ch + 1) * 1024)
                    for cb in range(8):
                        nc.sync.dma_start(
                            out=w1_sb[cb][:, cs],
                            in_=w1_d[cb * 128:(cb + 1) * 128, cs])
                    for fb in range(ch * 8, (ch + 1) * 8):
                        nc.sync.dma_start(out=w2_sb[fb][:],
                                          in_=w2_d[fb * 128:(fb + 1) * 128, :])
            load_x2(3)
            load_x2(4)
            ybf_p = bctx.enter_context(tc.tile_pool(name="ybf", bufs=4))
            yt_p = bctx.enter_context(tc.tile_pool(name="yt", bufs=16))
            z1_p = bctx.enter_context(tc.tile_pool(name="z1", bufs=6))
            u_p = bctx.enter_context(tc.tile_pool(name="u", bufs=2))
            psB = bctx.enter_context(tc.tile_pool(name="psB", bufs=2, space="PSUM"))

            preppedB = {}
            if FP8_FFN:
                preppedB[0] = (bridgeB["x2"], bridgeB["ybf"], bridgeB["yf8"])

            def prep_B(stx):
                """LN2 + y-transpose for supertile stx, emitted early so the
                in-order engine queues overlap it with the previous z-loop.
                With FP8_FFN the transposed y ships as 16x-scaled fp8 k-pairs
                (DoubleRow layout) straight from the transpose PSUM."""
                if stx >= NB or stx in preppedB:
                    return
                x2t = bridgeB["x2"] if stx == 0 else x2_tiles.pop(stx)
                ybf = [ybf_p.tile([128, C], BF16, tag="ybf", name="ybf")
                       for _ in range(2)]
                for tb in range(2):
                    _ln_tile(nc, pools, x2t[tb], ybf[tb],
                             (cb_t["g2b"], cb_t["be2b"]) if aff2 else None)
                ytT = []
                if FP8_FFN:
                    for cbp in range(4):
                        ytT.append(yt_p.tile([128, 2 * 256], F8E4, tag="yt8",
                                             name="yt8"))
                for cb in range(8):
                    pt = psB.tile([128, 256], BF16, tag="pt", name="pt",
                                  bufs=1)
                    for tb in range(2):
                        nc.tensor.transpose(
                            pt[:, tb * 128:(tb + 1) * 128],
                            ybf[tb][:, cb * 128:(cb + 1) * 128], idb[:])
                    if FP8_FFN:
                        nc.scalar.mul(
                            ytT[cb // 2][:, (cb % 2) * 256:(cb % 2) * 256 + 256],
                            pt[:], 16.0)
                    else:
                        t = yt_p.tile([128, 256], BF16, tag="yt", name="yt")
                        if cb % 2 == 0:
                            nc.vector.tensor_copy(t[:], pt[:])
                        else:
                            nc.scalar.copy(t[:], pt[:])
                        ytT.append(t)
                preppedB[stx] = (x2t, ybf, ytT)

            prep_B(0)

            for stx in range(NB):
                x2t, ybf, ytT = preppedB.pop(stx)
                z2ps = [psB.tile([128, 512], F32, tag="acc", name="acc", bufs=4)
                        for _ in range(4)]
                for fb in range(32):
                    if fb == 10:
                        load_x2(stx + 2)
                        prep_B(stx + 1)
                    pz = psB.tile([128, 256], F32, tag="pz", name="pz",
                                  bufs=3)
                    if FP8_FFN:
                        fs = (fb % 8) * 128
                        for cbp in range(4):
                            w1s = w1_oct[fb // 8][cbp][:].rearrange(
                                "p (j n) -> p j n", j=2)[:, :, fs:fs + 128]
                            yv = ytT[cbp][:].rearrange("p (j n) -> p j n", j=2)
                            nc.tensor.matmul(
                                pz[:], w1s, yv,
                                perf_mode=mybir.MatmulPerfMode.DoubleRow,
                                start=(cbp == 0), stop=(cbp == 3))
                    else:
                        for cb in range(8):
                            nc.tensor.matmul(
                                pz[:], w1_sb[cb][:, fb * 128:(fb + 1) * 128],
                                ytT[cb][:], start=(cb == 0), stop=(cb == 7))
                    z1 = z1_p.tile([128, 256], BF16, tag="z1", name="z1")
                    zsc = 1.0 / 256.0 if FP8_FFN else 1.0
                    if use_b1:
                        nc.scalar.activation(z1[:], pz[:], AF.Relu,
                                             bias=b1t[:, fb:fb + 1], scale=zsc)
                    else:
                        nc.scalar.activation(z1[:], pz[:], AF.Relu, scale=zsc)
                    for tb in range(2):
                        for n in range(2):
                            nc.tensor.matmul(
                                z2ps[tb * 2 + n][:],
                                z1[:, tb * 128:(tb + 1) * 128],
                                w2_sb[fb][:, n * 512:(n + 1) * 512],
                                start=(fb == 0), stop=(fb == 31),
                                skip_group_check=True)
                for tb in range(2):
                    u = u_p.tile([128, C], F32, tag="u", name="u")
                    for n in range(2):
                        nsl = slice(n * 512, (n + 1) * 512)
                        nc.vector.tensor_tensor(u[:, nsl], z2ps[tb * 2 + n][:],
                                                ybf[tb][:, nsl], ALU.add)
                    if use_b2:
                        nc.vector.tensor_tensor(u[:], u[:], cb_t["b2b"][:],
                                                ALU.add)
                    _ln_tile(nc, pools, u, u,
                             (cb_t["g3b"], cb_t["be3b"]) if aff3 else None)
                    row = stx * 256 + tb * 128
                    # half-column add+DMA: the output DMA overlaps the second
                    # half's add (shaves the drain tail, last supertile most)
                    for hh in range(2):
                        cs = slice(hh * 512, (hh + 1) * 512)
                        nc.vector.tensor_tensor(x2t[tb][:, cs], x2t[tb][:, cs],
                                                u[:, cs], ALU.add)
                        nc.sync.dma_start(out=out_d[row:row + 128, cs],
                                          in_=x2t[tb][:, cs])
    nc.finalize()
    return nc


def _get_nc(flags):
    key = ("nc", flags)
    if key not in _CACHE:
        _CACHE[key] = _build(flags)
    return _CACHE[key]


def kernel(x, wq, wk, wv, w_proj, b_proj, w1, b1, w2, b2,
           g1, be1, g2, be2, g3, be3):
    bf = ml_dtypes.bfloat16
    x = np.asarray(x, np.float32)

    def nz(v):
        return bool(np.any(np.asarray(v, np.float32) != 0.0))

    def naff(g, be):
        return bool(np.any(np.asarray(g, np.float32) != 1.0)) or nz(be)

    flags = (naff(g1, be1), naff(g2, be2), naff(g3, be3),
             nz(b_proj), nz(b1), nz(b2))
    aff1, aff2, aff3, use_bproj, use_b1, use_b2 = flags
    nc = _get_nc(flags)

    def bc(vec):
        return np.ascontiguousarray(
            np.broadcast_to(np.asarray(vec, np.float32).reshape(1, C),
                            (128, C))).astype(bf)

    f8 = mybir.dt.np(F8E4)

    def packqk(w):
        flat = np.asarray(w, np.float32).transpose(1, 0, 2).reshape(C, C)
        return np.ascontiguousarray(
            (16.0 * flat).reshape(4, 2, 128, C)
            .transpose(0, 2, 1, 3).reshape(C // 2, 2 * C)).astype(f8)

    wqv = packqk(wq)
    wkv = packqk(wk)
    wvf = np.ascontiguousarray(
        np.asarray(wv, np.float32).transpose(1, 0, 2).reshape(C, C)).astype(bf)
    wpf = np.asarray(w_proj, np.float32).astype(bf)
    if FP8_FFN:
        w1f = np.ascontiguousarray(
            (16.0 * np.asarray(w1, np.float32)).reshape(4, 2, 128, F4)
            .transpose(0, 2, 1, 3).reshape(C // 2, 2 * F4)).astype(f8)
    else:
        w1f = np.asarray(w1, np.float32).astype(bf)
    w2f = np.asarray(w2, np.float32).astype(bf)
    s = np.arange(128)[:, None]
    t = np.arange(128)[None, :]
    m0 = (s <= t).astype(np.float32).astype(bf)
    common = {
        "wqv": wqv, "wkv": wkv, "wvf": wvf, "wpf": wpf,
        ("w1q" if FP8_FFN else "w1f"): w1f, "w2f": w2f,
        "mask0": m0,
        "identb": np.eye(128, dtype=np.float32).astype(bf),
    }
    if use_b1:
        common["b1t"] = np.ascontiguousarray(
            np.asarray(b1, np.float32).reshape(F4 // 128, 128).T)
    if use_bproj:
        common["bprojb"] = bc(b_proj)
    if use_b2:
        common["b2b"] = bc(b2)
    if aff1:
        common["g1b"] = bc(g1)
        common["be1b"] = bc(be1)
    if aff2:
        common["g2b"] = bc(g2)
        common["be2b"] = bc(be2)
    if aff3:
        common["g3b"] = bc(g3)
        common["be3b"] = bc(be3)
    xs = x.reshape(NCORES, TOK, C).astype(bf)
    in_maps = [dict(common, x=np.ascontiguousarray(xs[i]))
               for i in range(NCORES)]
    import os
    trace = bool(os.environ.get("KERNEL_TRACE"))
    res = run_bass_kernel_spmd(nc, in_maps, core_ids=list(range(NCORES)),
                               trace=trace)
    _CACHE["last_res"] = res
    out = np.stack([res.results[i]["out"] for i in range(NCORES)], axis=0)
    return out.reshape(B, T, C).astype(np.float32)



# revision 52
# speedup vs baseline: 1.0080x; 1.0080x over previous
"""Trainium2 Bass kernel for a dense transformer block (B=64,T=256,C=1024,H=16).

Sharding: pure data-parallel over batch across 8 NeuronCores (8 sequences
per core, no collectives). Per-core program:
  phase A (per batch PAIR): LN1 (bn_stats, x ships bf16) -> h ->
  PE-transpose -> hT as bf16 (for V) and 16x-scaled fp8 k-pairs (for
  q/k); q/k via fp8 DoubleRow matmuls over both batches (N=512, PSUM
  drained on the scalar engine), V/proj in bf16; causal attention with
  both score blocks of a head packed into one PSUM bank; V stored
  head-interleaved as [V_h | 1] 65-col blocks so attn@V's 65th output
  column IS the softmax denominator (no N=1 ones-matmuls); exp on
  scalar, diag masks on vector (e0) / gpsimd (e1); normalization via
  per-partition vector multiplies -> PE-transpose cat back to [c,t] ->
  proj -> residual -> spill x2 to DRAM (batch 0 stays SBUF-resident).
  phase B (per 256-token supertile): LN2 -> yT emitted directly as
  16x-scaled fp8 k-pairs -> z1 = relu(w1.T y / 256) with w1 in fp8
  DoubleRow (halves GEMM1's PE passes; error budget: ~1.1e-2
  scale-relative absmax vs the 2e-2 gate; a both-GEMM fp8 FFN lands
  ~1.5e-2 - too close) -> z2 accumulation in bf16 (N=512 matmuls)
  -> LN3 -> final residual -> out.
  A top-level "bridge" pool holds supertile 0's x2/LN2/yT-fp8 and the
  first FFN weight chunk, all produced during phase A, so phase B's
  first pz matmul issues the moment the phase-A PSUM pool releases
  (pool-boundary serialization otherwise costs ~12us); the remaining
  w1 streams in fb-octet column chunks interleaved with the w2 rows
  they pair with.  LN prep and input loads software-pipeline one
  iteration ahead in both phases.
"""

from contextlib import ExitStack

import ml_dtypes
import numpy as np

import concourse.bass as bass
import concourse.bacc as bacc
import concourse.mybir as mybir
import concourse.tile as tile
from concourse.bass_utils import run_bass_kernel_spmd

F32 = mybir.dt.float32
BF16 = mybir.dt.bfloat16
F8E4 = mybir.dt.float8e4
AF = mybir.ActivationFunctionType
ALU = mybir.AluOpType
AX = mybir.AxisListType

B, T, C, H, D = 64, 256, 1024, 16, 64
FP8_FFN = True            # FFN first GEMM in fp8 DoubleRow (2x PE rate)
NCORES = 8
NB = B // NCORES          # 8 sequences per core
TOK = NB * T              # 2048 tokens per core
F4 = 4 * C                # 4096
EPS = 1e-3
SCALE = C ** -0.5         # 1/32

_CACHE = {}


def _ln_tile(nc, pools, xt, out_t, affine, ncols=C, scale32=False,
             ts_eng=None):
    """LayerNorm of one [128, ncols] tile along the free axis via bn_stats.
    out_t may equal xt (in-place). affine = (g_t, be_t) or None.
    scale32: emit 32*(x-m)*rstd by folding 1/1024 into the sqrt.
    ts_eng: engine for the big normalize pass (e.g. nc.gpsimd when both
    xt and out_t are SBUF tiles); defaults to the vector engine."""
    stat = pools["stat"]
    nsub = ncols // 512
    st = stat.tile([128, nsub, 6], F32, tag="bst", name="bst")
    mv = stat.tile([128, 2], F32, tag="bmv", name="bmv")
    rs = stat.tile([128, 1], F32, tag="brs", name="brs")
    xv = xt[:].rearrange("p (a b) -> p a b", b=512)
    for i in range(nsub):
        nc.vector.bn_stats(st[:, i, :], xv[:, i, :])
    nc.vector.bn_aggr(mv[:], st[:])
    if scale32:
        nc.scalar.activation(rs[:], mv[:, 1:2], AF.Sqrt,
                             bias=pools["eps1024"][:], scale=1.0 / 1024.0)
    else:
        nc.scalar.activation(rs[:], mv[:, 1:2], AF.Sqrt, bias=pools["eps"][:])
    nc.vector.reciprocal(rs[:], rs[:])
    (ts_eng or nc.vector).tensor_scalar(out_t[:], xt[:], mv[:, 0:1], rs[:],
                                        ALU.subtract, ALU.mult)
    if affine is not None:
        g_t, be_t = affine
        nc.vector.tensor_tensor(out_t[:], out_t[:], g_t[:], ALU.mult)
        nc.vector.tensor_tensor(out_t[:], out_t[:], be_t[:], ALU.add)


def _build(flags):
    aff1, aff2, aff3, use_bproj, use_b1, use_b2 = flags
    nc = bacc.Bacc(target_bir_lowering=False)
    # x ships bf16: halves the xb pool + startup DMA; the trunk residual
    # only carries bf16 rounding of x (~4x below the fp8-FFN error).
    x_d = nc.dram_tensor("x", [TOK, C], BF16, kind="ExternalInput")
    # q/k projections ship as fp8e4, pre-scaled by 16 and packed in the
    # DoubleRow k-pair layout [C/2 rows, 2*C cols]; the 16*16*16*16 scale
    # surplus is divided out inside the softmax exp.
    wqv_d = nc.dram_tensor("wqv", [C // 2, 2 * C], F8E4, kind="ExternalInput")
    wkv_d = nc.dram_tensor("wkv", [C // 2, 2 * C], F8E4, kind="ExternalInput")
    wv_d = nc.dram_tensor("wvf", [C, C], BF16, kind="ExternalInput")
    wp_d = nc.dram_tensor("wpf", [C, C], BF16, kind="ExternalInput")
    if FP8_FFN:
        # w1 ships fp8e4 pre-scaled by 16 in DoubleRow k-pair layout,
        # like wq/wk; the 16*16 scale is divided out in the relu.
        w1_d = nc.dram_tensor("w1q", [C // 2, 2 * F4], F8E4,
                              kind="ExternalInput")
    else:
        w1_d = nc.dram_tensor("w1f", [C, F4], BF16, kind="ExternalInput")
    w2_d = nc.dram_tensor("w2f", [F4, C], BF16, kind="ExternalInput")
    consts_bf = {}
    names = []
    if use_b1:
        b1_d = nc.dram_tensor("b1t", [128, F4 // 128], F32, kind="ExternalInput")
    if use_bproj:
        names.append("bprojb")
    if use_b2:
        names.append("b2b")
    if aff1:
        names += ["g1b", "be1b"]
    if aff2:
        names += ["g2b", "be2b"]
    if aff3:
        names += ["g3b", "be3b"]
    for nm in names:
        consts_bf[nm] = nc.dram_tensor(nm, [128, C], BF16, kind="ExternalInput")
    m0_d = nc.dram_tensor("mask0", [128, 128], BF16, kind="ExternalInput")
    id_d = nc.dram_tensor("identb", [128, 128], BF16, kind="ExternalInput")
    out_d = nc.dram_tensor("out", [TOK, C], F32, kind="ExternalOutput")
    x2_d = nc.dram_tensor("x2d", [TOK, C], F32)

    with tile.TileContext(nc) as tc, ExitStack() as ctx:
        const = ctx.enter_context(tc.tile_pool(name="const", bufs=1))
        cb_t = {nm: const.tile([128, C], BF16, tag=nm, name=nm)
                for nm in consts_bf}
        for nm, t in cb_t.items():
            nc.sync.dma_start(out=t[:], in_=consts_bf[nm][:, :])
        m0 = const.tile([128, 128], BF16, tag="m0", name="m0")
        nc.sync.dma_start(out=m0[:], in_=m0_d[:, :])
        idb = const.tile([128, 128], BF16, tag="idb", name="idb")
        nc.sync.dma_start(out=idb[:], in_=id_d[:, :])
        if use_b1:
            b1t = const.tile([128, F4 // 128], F32, tag="b1t", name="b1t")
            nc.sync.dma_start(out=b1t[:], in_=b1_d[:, :])

        epsb = const.tile([128, 1], F32, tag="eps", name="eps")
        nc.gpsimd.memset(epsb[:], EPS)
        epsb2 = const.tile([128, 1], F32, tag="eps2", name="eps2")
        nc.gpsimd.memset(epsb2[:], EPS / 1024.0)

        stat = ctx.enter_context(tc.tile_pool(name="stat", bufs=8))
        pools = {"stat": stat, "eps": epsb, "eps1024": epsb2}

        # bridge pool: first FFN weight chunk lives at top level so its DMA
        # streams during phase A (phase-B pools can't allocate until the
        # phase-A pools release, serializing their DMAs behind all of A).
        bridge = ctx.enter_context(tc.tile_pool(name="bridge", bufs=1))
        w1_oct0, w2_pre = [], []
        if FP8_FFN:
            for cbp in range(4):
                t = bridge.tile([128, 2 * 1024], F8E4, tag=f"w1o0_{cbp}",
                                name=f"w1o0_{cbp}")
                w1_oct0.append(t)
        for fb in range(2):
            t = bridge.tile([128, C], BF16, tag=f"w2p{fb}", name=f"w2p{fb}")
            w2_pre.append(t)
        # supertile 0's x2 / LN2 / yT(fp8) also live in the bridge: its FFN
        # inputs are fully prepared inside phase A, so the first pz matmul
        # issues the moment the phase-A PSUM pool releases.
        bridgeB = {
            "x2": [bridge.tile([128, C], F32, tag=f"bx2_{tb}", name=f"bx2_{tb}")
                   for tb in range(2)],
            "ybf": [bridge.tile([128, C], BF16, tag=f"by_{tb}", name=f"by_{tb}")
                    for tb in range(2)],
            "yf8": [bridge.tile([128, 2 * 256], F8E4, tag=f"by8_{c}",
                                name=f"by8_{c}") for c in range(4)],
        }

        # ---------------- phase A: attention ----------------
        with ExitStack() as actx:
            xb_p = actx.enter_context(tc.tile_pool(name="xb", bufs=12))
            xb_tiles = {}

            def load_x(b):
                if b >= NB:
                    return
                ts = [xb_p.tile([128, C], BF16, tag="xb", name="xb")
                      for _ in range(2)]
                for tb in range(2):
                    row = b * T + tb * 128
                    nc.sync.dma_start(out=ts[tb][:], in_=x_d[row:row + 128, :])
                xb_tiles[b] = ts

            # first pair's input loads go on the queue before the weight
            # DMAs so LN1 can start while weights stream in; pair 1's loads
            # follow the weights (they aren't needed until ~30us in).
            load_x(0)
            load_x(1)

            wpool = actx.enter_context(tc.tile_pool(name="wqkv", bufs=1))
            wqv_sb, wkv_sb = [], []
            for cbp in range(4):
                for lst, dram, nm in ((wqv_sb, wqv_d, "wqv"),
                                      (wkv_sb, wkv_d, "wkv")):
                    t = wpool.tile([128, 2 * C], F8E4, tag=f"{nm}{cbp}",
                                   name=f"{nm}{cbp}")
                    nc.sync.dma_start(out=t[:],
                                      in_=dram[cbp * 128:(cbp + 1) * 128, :])
                    lst.append(t)
            wv_sb, wp_sb = [], []
            for cb in range(8):
                for lst, dram, nm in ((wv_sb, wv_d, "wv"), (wp_sb, wp_d, "wp")):
                    t = wpool.tile([128, C], BF16, tag=f"{nm}{cb}", name=f"{nm}{cb}")
                    nc.sync.dma_start(out=t[:], in_=dram[cb * 128:(cb + 1) * 128, :])
                    lst.append(t)
            load_x(2)
            load_x(3)
            h_p = actx.enter_context(tc.tile_pool(name="h", bufs=4))
            ht_p = actx.enter_context(tc.tile_pool(name="ht", bufs=16))
            hf8_p = actx.enter_context(tc.tile_pool(name="hf8", bufs=8))
            qt_p = actx.enter_context(tc.tile_pool(name="qt", bufs=12))
            kt_p = actx.enter_context(tc.tile_pool(name="kt", bufs=12))
            v_p = actx.enter_context(tc.tile_pool(name="v", bufs=8))
            ex_p = actx.enter_context(tc.tile_pool(name="ex", bufs=8))
            cat_p = actx.enter_context(tc.tile_pool(name="cat", bufs=4))
            ctt_p = actx.enter_context(tc.tile_pool(name="ctt", bufs=10))
            rec_p = actx.enter_context(tc.tile_pool(name="rec", bufs=12))
            x2_p = actx.enter_context(tc.tile_pool(name="x2", bufs=4))
            ps = actx.enter_context(tc.tile_pool(name="psA", bufs=8, space="PSUM"))

            prepped = {}

            def prep_A2(bp):
                """LN1 + h-transpose for a batch pair (emitted one pair early
                so the in-order vector/PE queues overlap it with pair bp-1).
                Produces hT both as bf16 (for V) and as 16x-scaled fp8 pairs
                (for the DoubleRow q/k matmuls)."""
                if bp >= NB // 2:
                    return
                b0, b1 = 2 * bp, 2 * bp + 1
                xbs = {b0: xb_tiles.pop(b0), b1: xb_tiles.pop(b1)}
                hbf = [h_p.tile([128, C], BF16, tag="h", name="h")
                       for _ in range(4)]
                for i in range(4):
                    _ln_tile(nc, pools, xbs[b0 if i < 2 else b1][i % 2],
                             hbf[i],
                             (cb_t["g1b"], cb_t["be1b"]) if aff1 else None)
                ht, hf8 = [], []
                for cbp in range(4):
                    hf8.append(hf8_p.tile([128, 2 * 512], F8E4, tag="hf8",
                                          name="hf8"))
                for cb in range(8):
                    pt = ps.tile([128, 512], BF16, tag="ps", name="ps")
                    for tt in range(4):
                        nc.tensor.transpose(
                            pt[:, tt * 128:(tt + 1) * 128],
                            hbf[tt][:, cb * 128:(cb + 1) * 128], idb[:])
                    t = ht_p.tile([128, 512], BF16, tag="ht", name="ht")
                    nc.vector.tensor_copy(t[:], pt[:])
                    ht.append(t)
                    nc.scalar.mul(
                        hf8[cb // 2][:, (cb % 2) * 512:(cb % 2) * 512 + 512],
                        pt[:], 16.0)
                prepped[bp] = (xbs, ht, hf8)

            prep_A2(0)

            for bp in range(NB // 2):
                xbs, ht, hf8 = prepped.pop(bp)
                hf8v = [t[:].rearrange("p (j n) -> p j n", j=2) for t in hf8]
                # q/k for both batches: fp8 DoubleRow, 256x-scaled outputs
                qt, kt = [], []
                for p in range(8):
                    pq = ps.tile([128, 512], F32, tag="ps", name="ps")
                    pk = ps.tile([128, 512], F32, tag="ps", name="ps")
                    for cbp in range(4):
                        wqs = wqv_sb[cbp][:].rearrange(
                            "p (j n) -> p j n", j=2)[:, :, p * 128:(p + 1) * 128]
                        wks = wkv_sb[cbp][:].rearrange(
                            "p (j n) -> p j n", j=2)[:, :, p * 128:(p + 1) * 128]
                        nc.tensor.matmul(
                            pq[:], wqs, hf8v[cbp],
                            perf_mode=mybir.MatmulPerfMode.DoubleRow,
                            start=(cbp == 0), stop=(cbp == 3))
                        nc.tensor.matmul(
                            pk[:], wks, hf8v[cbp],
                            perf_mode=mybir.MatmulPerfMode.DoubleRow,
                            start=(cbp == 0), stop=(cbp == 3))
                    tq = qt_p.tile([128, 512], BF16, tag="qt", name="qt")
                    tk = kt_p.tile([128, 512], BF16, tag="kt", name="kt")
                    # kt drains on scalar: vector is the scarce engine during
                    # the attention/prep overlap windows (qt stays on vector;
                    # both on scalar starves the exp stream).
                    nc.vector.tensor_copy(tq[:], pq[:])
                    nc.scalar.copy(tk[:], pk[:])
                    qt.append(tq)
                    kt.append(tk)
                # V per batch (bf16), stored head-interleaved as [V_h | 1]
                # blocks of 65 cols so attn@V emits the softmax denominator
                # as a fused 65th output column (kills the N=1 ones-matmuls).
                vsbs = {}
                for bi, b in enumerate((2 * bp, 2 * bp + 1)):
                    vsb = []
                    for sb in range(2):
                        scol = (bi * 2 + sb) * 128
                        pv = [ps.tile([128, 512], F32, tag="ps", name="ps")
                              for _ in range(2)]
                        for cb in range(8):
                            for q4 in range(4):
                                nc.tensor.matmul(
                                    pv[q4 // 2][:, (q4 % 2) * 256:(q4 % 2) * 256 + 256],
                                    ht[cb][:, scol:scol + 128],
                                    wv_sb[cb][:, q4 * 256:(q4 + 1) * 256],
                                    start=(cb == 0 and q4 % 2 == 0),
                                    stop=(cb == 7 and q4 % 2 == 1),
                                    skip_group_check=True)
                        tv = v_p.tile([128, 16 * 65], BF16, tag="v", name="v")
                        tv3 = tv[:].rearrange("p (h x) -> p h x", x=65)
                        nc.gpsimd.memset(tv3[:, :, 64], 1.0)
                        nc.vector.tensor_copy(
                            tv3[:, 0:8, 0:64],
                            pv[0][:].rearrange("p (h x) -> p h x", x=64))
                        nc.vector.tensor_copy(
                            tv3[:, 8:16, 0:64],
                            pv[1][:].rearrange("p (h x) -> p h x", x=64))
                        vsb.append(tv)
                    vsbs[b] = vsb
                # attention + proj per batch
                for bi, b in enumerate((2 * bp, 2 * bp + 1)):
                    vsb = vsbs[b]
                    bcol = bi * 256
                    cat_t = [cat_p.tile([128, C], BF16, tag="cat", name="cat")
                             for _ in range(2)]
                    for pr in range(8):
                        if bi == 1 and pr == 2:
                            load_x(2 * bp + 4)
                            load_x(2 * bp + 5)
                            prep_A2(bp + 1)
                        # both score blocks of one head packed in one PSUM
                        # bank (cols 0:256 = s-block0 x both tb, 256:384 =
                        # s-block1 x tb1) -> 3 ring slots per pr instead of 5.
                        scps, e0s, e1s = [], [], []
                        for off in (0, 64):
                            qs = qt[pr][off:off + 64, bcol:bcol + 256]
                            ks = kt[pr][off:off + 64, bcol:bcol + 256]
                            scp = ps.tile([128, 384], F32, tag="ps", name="scp")
                            nc.tensor.matmul(scp[:, 0:256], ks[:, 0:128],
                                             qs[:], start=True, stop=False,
                                             skip_group_check=True)
                            nc.tensor.matmul(scp[:, 256:384], ks[:, 128:256],
                                             qs[:, 128:256], start=False,
                                             stop=True, skip_group_check=True)
                            scps.append(scp)
                        for i in range(2):
                            e0 = ex_p.tile([128, 256], BF16, tag="e0", name="e0")
                            e1 = ex_p.tile([128, 128], BF16, tag="e1", name="e1")
                            nc.scalar.activation(e0[:], scps[i][:, 0:256],
                                                 AF.Exp, scale=SCALE / 65536.0)
                            nc.scalar.activation(e1[:], scps[i][:, 256:384],
                                                 AF.Exp, scale=SCALE / 65536.0)
                            # e0's mask feeds the first attn@V matmul: keep it
                            # on vector (short latency); e1 has more slack and
                            # keeps gpsimd from idling.
                            nc.vector.tensor_tensor(e0[:, 0:128], e0[:, 0:128],
                                                    m0[:], ALU.mult)
                            nc.gpsimd.tensor_tensor(e1[:], e1[:], m0[:],
                                                    ALU.mult)
                            e0s.append(e0)
                            e1s.append(e1)
                        # attn@V for BOTH heads packed in one bank as four
                        # 65-col blocks [out_h | den]: the denominator rides
                        # along as V's interleaved ones column.  One
                        # accumulation chain across all 6 matmuls.
                        att = ps.tile([128, 260], F32, tag="ps", name="att")
                        for i in range(2):
                            hh = 2 * pr + i
                            hs65 = slice(hh * 65, hh * 65 + 65)
                            e0, e1 = e0s[i], e1s[i]
                            o = 130 * i
                            nc.tensor.matmul(att[:, o:o + 65], e0[:, 0:128],
                                             vsb[0][:, hs65], start=(i == 0),
                                             stop=False, skip_group_check=True)
                            nc.tensor.matmul(att[:, o + 65:o + 130],
                                             e0[:, 128:256], vsb[0][:, hs65],
                                             start=False, stop=False,
                                             skip_group_check=True)
                            nc.tensor.matmul(att[:, o + 65:o + 130], e1[:],
                                             vsb[1][:, hs65], start=False,
                                             stop=(i == 1),
                                             skip_group_check=True)
                        # one reciprocal over all four denominators (strided
                        # col 64 of each 65-block): RAW on the last matmul of
                        # the bank, so the norm muls follow all PE writes.
                        av = att[:].rearrange("p (a b) -> p a b", b=65)
                        rec = rec_p.tile([128, 4], F32, tag="rec", name="rec")
                        nc.vector.reciprocal(rec[:], av[:, :, 64])
                        for i in range(2):
                            hh = 2 * pr + i
                            hs = slice(hh * 64, (hh + 1) * 64)
                            o = 130 * i
                            nc.vector.tensor_scalar_mul(
                                cat_t[0][:, hs], att[:, o:o + 64],
                                rec[:, 2 * i:2 * i + 1])
                            nc.vector.tensor_scalar_mul(
                                cat_t[1][:, hs], att[:, o + 65:o + 129],
                                rec[:, 2 * i + 1:2 * i + 2])
                    # transpose cat_t -> catT [c, t]
                    catT = []
                    for cb in range(8):
                        pt = ps.tile([128, 256], BF16, tag="ps", name="ps")
                        for tb in range(2):
                            nc.tensor.transpose(
                                pt[:, tb * 128:(tb + 1) * 128],
                                cat_t[tb][:, cb * 128:(cb + 1) * 128], idb[:])
                        t = ctt_p.tile([128, 256], BF16, tag="ctt", name="ctt")
                        nc.vector.tensor_copy(t[:], pt[:])
                        catT.append(t)
                    # proj + residual -> x2 -> DRAM spill (batch 0 keeps its
                    # x2 SBUF-resident in the bridge; no round-trip).
                    xb = xbs[b]
                    for tb in range(2):
                        if b == 0:
                            x2t = bridgeB["x2"][tb]
                        else:
                            x2t = x2_p.tile([128, C], F32, tag="x2", name="x2")
                        for n in range(2):
                            pp = ps.tile([128, 512], F32, tag="ps", name="ps")
                            for cb in range(8):
                                for nh in range(2):
                                    nc.tensor.matmul(
                                        pp[:, nh * 256:(nh + 1) * 256],
                                        catT[cb][:, tb * 128:(tb + 1) * 128],
                                        wp_sb[cb][:, n * 512 + nh * 256:
                                                   n * 512 + (nh + 1) * 256],
                                        start=(cb == 0 and nh == 0),
                                        stop=(cb == 7 and nh == 1),
                                        skip_group_check=True)
                            nsl = slice(n * 512, (n + 1) * 512)
                            nc.vector.tensor_tensor(x2t[:, nsl], pp[:],
                                                    xb[tb][:, nsl], ALU.add)
                            if use_bproj:
                                nc.vector.tensor_tensor(
                                    x2t[:, nsl], x2t[:, nsl],
                                    cb_t["bprojb"][:, nsl], ALU.add)
                        if b != 0:
                            row = b * T + tb * 128
                            nc.sync.dma_start(out=x2_d[row:row + 128, :],
                                              in_=x2t[:])

            # bridge weight DMAs: emitted last so they yield DMA priority to
            # phase A's own traffic, but stream well before the boundary.
            if FP8_FFN:
                for cbp in range(4):
                    for j in range(2):
                        nc.sync.dma_start(
                            out=w1_oct0[cbp][:, j * 1024:(j + 1) * 1024],
                            in_=w1_d[cbp * 128:(cbp + 1) * 128,
                                     j * F4:j * F4 + 1024])
            for fb in range(2):
                nc.sync.dma_start(out=w2_pre[fb][:],
                                  in_=w2_d[fb * 128:(fb + 1) * 128, :])

            # prep supertile 0's FFN inputs inside phase A (LN2 on vector,
            # transposes through the phase-A PSUM ring, fp8 via scalar).
            for tb in range(2):
                _ln_tile(nc, pools, bridgeB["x2"][tb], bridgeB["ybf"][tb],
                         (cb_t["g2b"], cb_t["be2b"]) if aff2 else None)
            for cb in range(8):
                pt0 = ps.tile([128, 256], BF16, tag="ps", name="pt0")
                for tb in range(2):
                    nc.tensor.transpose(
                        pt0[:, tb * 128:(tb + 1) * 128],
                        bridgeB["ybf"][tb][:, cb * 128:(cb + 1) * 128], idb[:])
                nc.scalar.mul(
                    bridgeB["yf8"][cb // 2][:, (cb % 2) * 256:(cb % 2) * 256 + 256],
                    pt0[:], 16.0)

        # ---------------- phase B: FFN ----------------
        with ExitStack() as bctx:
            x2B_p = bctx.enter_context(tc.tile_pool(name="x2B", bufs=12))
            x2_tiles = {}

            def load_x2(stx):
                # stx 0 is SBUF-resident via the bridge; guard duplicates
                # (the steady-state prefetch revisits early indices).
                if stx >= NB or stx == 0 or stx in x2_tiles or stx in _x2_seen:
                    return
                _x2_seen.add(stx)
                ts = [x2B_p.tile([128, C], F32, tag="x2B", name="x2B")
                      for _ in range(2)]
                for tb in range(2):
                    row = stx * 256 + tb * 128
                    nc.sync.dma_start(out=ts[tb][:], in_=x2_d[row:row + 128, :])
                x2_tiles[stx] = ts

            _x2_seen = set()
            # first supertiles' loads precede the FFN weight DMAs on the queue
            load_x2(1)
            load_x2(2)

            wpoolB = bctx.enter_context(tc.tile_pool(name="wffn", bufs=1))
            # w1q lives as per-fb-octet tiles: octet 0 is the bridge pool's
            # (DMA'd during phase A); octets 1-3 stream here, interleaved
            # with the w2 rows they pair with.
            w1_oct, w2_sb = [w1_oct0], list(w2_pre)
            for fb in range(2, 32):
                t = wpoolB.tile([128, C], BF16, tag=f"w2_{fb}", name=f"w2_{fb}")
                w2_sb.append(t)
            if FP8_FFN:
                for oc in range(1, 4):
                    tiles = []
                    for cbp in range(4):
                        t = wpoolB.tile([128, 2 * 1024], F8E4,
                                        tag=f"w1o{oc}_{cbp}",
                                        name=f"w1o{oc}_{cbp}")
                        tiles.append(t)
                    w1_oct.append(tiles)
                for fb in range(2, 8):
                    nc.sync.dma_start(out=w2_sb[fb][:],
                                      in_=w2_d[fb * 128:(fb + 1) * 128, :])
                for oc in range(1, 4):
                    for cbp in range(4):
                        for j in range(2):
                            nc.sync.dma_start(
                                out=w1_oct[oc][cbp][:, j * 1024:(j + 1) * 1024],
                                in_=w1_d[cbp * 128:(cbp + 1) * 128,
                                         j * F4 + oc * 1024:
                                         j * F4 + (oc + 1) * 1024])
                    for fb in range(oc * 8, (oc + 1) * 8):
                        nc.sync.dma_start(out=w2_sb[fb][:],
                                          in_=w2_d[fb * 128:(fb + 1) * 128, :])
            else:
                w1_sb = []
                for cb in range(8):
                    t = wpoolB.tile([128, F4], BF16, tag=f"w1_{cb}",
                                    name=f"w1_{cb}")
                    w1_sb.append(t)
                for ch in range(4):
                    cs = slice(ch * 1024, (ch + 1) * 1024)
                    for cb in range(8):
                        nc.sync.dma_start(
                            out=w1_sb[cb][:, cs],
                            in_=w1_d[cb * 128:(cb + 1) * 128, cs])
                    for fb in range(ch * 8, (ch + 1) * 8):
                        nc.sync.dma_start(out=w2_sb[fb][:],
                                          in_=w2_d[fb * 128:(fb + 1) * 128, :])
            load_x2(3)
            load_x2(4)
            ybf_p = bctx.enter_context(tc.tile_pool(name="ybf", bufs=4))
            yt_p = bctx.enter_context(tc.tile_pool(name="yt", bufs=16))
            z1_p = bctx.enter_context(tc.tile_pool(name="z1", bufs=6))
            u_p = bctx.enter_context(tc.tile_pool(name="u", bufs=2))
            psB = bctx.enter_context(tc.tile_pool(name="psB", bufs=2, space="PSUM"))

            preppedB = {}
            if FP8_FFN:
                preppedB[0] = (bridgeB["x2"], bridgeB["ybf"], bridgeB["yf8"])

            def prep_B(stx):
                """LN2 + y-transpose for supertile stx, emitted early so the
                in-order engine queues overlap it with the previous z-loop.
                With FP8_FFN the transposed y ships as 16x-scaled fp8 k-pairs
                (DoubleRow layout) straight from the transpose PSUM."""
                if stx >= NB or stx in preppedB:
                    return
                x2t = bridgeB["x2"] if stx == 0 else x2_tiles.pop(stx)
                ybf = [ybf_p.tile([128, C], BF16, tag="ybf", name="ybf")
                       for _ in range(2)]
                for tb in range(2):
                    _ln_tile(nc, pools, x2t[tb], ybf[tb],
                             (cb_t["g2b"], cb_t["be2b"]) if aff2 else None)
                ytT = []
                if FP8_FFN:
                    for cbp in range(4):
                        ytT.append(yt_p.tile([128, 2 * 256], F8E4, tag="yt8",
                                             name="yt8"))
                for cb in range(8):
                    pt = psB.tile([128, 256], BF16, tag="pt", name="pt",
                                  bufs=1)
                    for tb in range(2):
                        nc.tensor.transpose(
                            pt[:, tb * 128:(tb + 1) * 128],
                            ybf[tb][:, cb * 128:(cb + 1) * 128], idb[:])
                    if FP8_FFN:
                        nc.scalar.mul(
                            ytT[cb // 2][:, (cb % 2) * 256:(cb % 2) * 256 + 256],
                            pt[:], 16.0)
                    else:
                        t = yt_p.tile([128, 256], BF16, tag="yt", name="yt")
                        if cb % 2 == 0:
                            nc.vector.tensor_copy(t[:], pt[:])
                        else:
                            nc.scalar.copy(t[:], pt[:])
                        ytT.append(t)
                preppedB[stx] = (x2t, ybf, ytT)

            prep_B(0)

            for stx in range(NB):
                x2t, ybf, ytT = preppedB.pop(stx)
                z2ps = [psB.tile([128, 512], F32, tag="acc", name="acc", bufs=4)
                        for _ in range(4)]
                for fb in range(32):
                    if fb == 10:
                        load_x2(stx + 2)
                        prep_B(stx + 1)
                    pz = psB.tile([128, 256], F32, tag="pz", name="pz",
                                  bufs=3)
                    if FP8_FFN:
                        fs = (fb % 8) * 128
                        for cbp in range(4):
                            w1s = w1_oct[fb // 8][cbp][:].rearrange(
                                "p (j n) -> p j n", j=2)[:, :, fs:fs + 128]
                            yv = ytT[cbp][:].rearrange("p (j n) -> p j n", j=2)
                            nc.tensor.matmul(
                                pz[:], w1s, yv,
                                perf_mode=mybir.MatmulPerfMode.DoubleRow,
                                start=(cbp == 0), stop=(cbp == 3))
                    else:
                        for cb in range(8):
                            nc.tensor.matmul(
                                pz[:], w1_sb[cb][:, fb * 128:(fb + 1) * 128],
                                ytT[cb][:], start=(cb == 0), stop=(cb == 7))
                    z1 = z1_p.tile([128, 256], BF16, tag="z1", name="z1")
                    zsc = 1.0 / 256.0 if FP8_FFN else 1.0
                    if use_b1:
                        nc.scalar.activation(z1[:], pz[:], AF.Relu,
                                             bias=b1t[:, fb:fb + 1], scale=zsc)
                    else:
                        nc.scalar.activation(z1[:], pz[:], AF.Relu, scale=zsc)
                    for tb in range(2):
                        for n in range(2):
                            nc.tensor.matmul(
                                z2ps[tb * 2 + n][:],
                                z1[:, tb * 128:(tb + 1) * 128],
                                w2_sb[fb][:, n * 512:(n + 1) * 512],
                                start=(fb == 0), stop=(fb == 31),
                                skip_group_check=True)
                for tb in range(2):
                    u = u_p.tile([128, C], F32, tag="u", name="u")
                    for n in range(2):
                        nsl = slice(n * 512, (n + 1) * 512)
                        nc.vector.tensor_tensor(u[:, nsl], z2ps[tb * 2 + n][:],
                                                ybf[tb][:, nsl], ALU.add)
                    if use_b2:
                        nc.vector.tensor_tensor(u[:], u[:], cb_t["b2b"][:],
                                                ALU.add)
                    _ln_tile(nc, pools, u, u,
                             (cb_t["g3b"], cb_t["be3b"]) if aff3 else None)
                    nc.vector.tensor_tensor(x2t[tb][:], x2t[tb][:], u[:],
                                            ALU.add)
                    row = stx * 256 + tb * 128
                    nc.sync.dma_start(out=out_d[row:row + 128, :],
                                      in_=x2t[tb][:])
    nc.finalize()
    return nc


def _get_nc(flags):
    key = ("nc", flags)
    if key not in _CACHE:
        _CACHE[key] = _build(flags)
    return _CACHE[key]


def kernel(x, wq, wk, wv, w_proj, b_proj, w1, b1, w2, b2,
           g1, be1, g2, be2, g3, be3):
    bf = ml_dtypes.bfloat16
    x = np.asarray(x, np.float32)

    def nz(v):
        return bool(np.any(np.asarray(v, np.float32) != 0.0))

    def naff(g, be):
        return bool(np.any(np.asarray(g, np.float32) != 1.0)) or nz(be)

    flags = (naff(g1, be1), naff(g2, be2), naff(g3, be3),
             nz(b_proj), nz(b1), nz(b2))
    aff1, aff2, aff3, use_bproj, use_b1, use_b2 = flags
    nc = _get_nc(flags)

    def bc(vec):
        return np.ascontiguousarray(
            np.broadcast_to(np.asarray(vec, np.float32).reshape(1, C),
                            (128, C))).astype(bf)

    f8 = mybir.dt.np(F8E4)

    def packqk(w):
        flat = np.asarray(w, np.float32).transpose(1, 0, 2).reshape(C, C)
        return np.ascontiguousarray(
            (16.0 * flat).reshape(4, 2, 128, C)
            .transpose(0, 2, 1, 3).reshape(C // 2, 2 * C)).astype(f8)

    wqv = packqk(wq)
    wkv = packqk(wk)
    wvf = np.ascontiguousarray(
        np.asarray(wv, np.float32).transpose(1, 0, 2).reshape(C, C)).astype(bf)
    wpf = np.asarray(w_proj, np.float32).astype(bf)
    if FP8_FFN:
        w1f = np.ascontiguousarray(
            (16.0 * np.asarray(w1, np.float32)).reshape(4, 2, 128, F4)
            .transpose(0, 2, 1, 3).reshape(C // 2, 2 * F4)).astype(f8)
    else:
        w1f = np.asarray(w1, np.float32).astype(bf)
    w2f = np.asarray(w2, np.float32).astype(bf)
    s = np.arange(128)[:, None]
    t = np.arange(128)[None, :]
    m0 = (s <= t).astype(np.float32).astype(bf)
    common = {
        "wqv": wqv, "wkv": wkv, "wvf": wvf, "wpf": wpf,
        ("w1q" if FP8_FFN else "w1f"): w1f, "w2f": w2f,
        "mask0": m0,
        "identb": np.eye(128, dtype=np.float32).astype(bf),
    }
    if use_b1:
        common["b1t"] = np.ascontiguousarray(
            np.asarray(b1, np.float32).reshape(F4 // 128, 128).T)
    if use_bproj:
        common["bprojb"] = bc(b_proj)
    if use_b2:
        common["b2b"] = bc(b2)
    if aff1:
        common["g1b"] = bc(g1)
        common["be1b"] = bc(be1)
    if aff2:
        common["g2b"] = bc(g2)
        common["be2b"] = bc(be2)
    if aff3:
        common["g3b"] = bc(g3)
        common["be3b"] = bc(be3)
    xs = x.reshape(NCORES, TOK, C).astype(bf)
    in_maps = [dict(common, x=np.ascontiguousarray(xs[i]))
               for i in range(NCORES)]
    import os
    trace = bool(os.environ.get("KERNEL_TRACE"))
    res = run_bass_kernel_spmd(nc, in_maps, core_ids=list(range(NCORES)),
                               trace=trace)
    _CACHE["last_res"] = res
    out = np.stack([res.results[i]["out"] for i in range(NCORES)], axis=0)
    return out.reshape(B, T, C).astype(np.float32)

